# revision 1
# baseline (speedup 1.0000x reference)
"""Data-parallel 8-core Trainium kernel for the 3-layer atom-embedding
message-passing block.

Strategy (per sharding hint): shard the flattened point dimension
B*N = 400000 across the 8 NeuronCores (50000 points each). All params
(<1KB) are replicated. GroupNorm is per point, so the cores never
communicate; each core runs the full 3-layer update on its point shard
and the host concatenates the shards back into the full output.
"""
import numpy as np
import jax
import jax.numpy as jnp
from functools import partial

B, N, K, D = 4, 100000, 16, 6
F = 2 * D + 1  # 13
L = 3
EPS = 1e-5
SLOPE = 0.2
NCORES = 8
PTS = B * N // NCORES  # 50000 points per core


@partial(jax.pmap, axis_name="c")
def _run_shard(atom, dist, w1, b1, w2, b2, gw, gb):
    # atom: [PTS, K, D], dist: [PTS, K, 1] on each core
    n = atom.shape[0]
    pe = jnp.ones((n, D), dtype=atom.dtype)
    for i in range(L):
        feat = jnp.concatenate(
            [jnp.broadcast_to(pe[:, None, :], (n, K, D)), atom, dist], axis=-1
        )
        h = jax.nn.leaky_relu(feat @ w1[i] + b1[i], SLOPE)
        messages = (h @ w2[i] + b2[i]).sum(-2)  # [n, D]
        g = messages.reshape(n, 2, 3)
        mu = g.mean(-1, keepdims=True)
        var = ((g - mu) ** 2).mean(-1, keepdims=True)
        xn = ((g - mu) * jax.lax.rsqrt(var + EPS)).reshape(n, D)
        normed = xn * gw[i] + gb[i]
        pe = pe + jax.nn.leaky_relu(normed, SLOPE)
    return pe


def kernel(dist, atomtypes, mlp_w1, mlp_b1, mlp_w2, mlp_b2, gn_w, gn_b):
    dist = np.asarray(dist, dtype=np.float32)
    atomtypes = np.asarray(atomtypes, dtype=np.float32)
    # shard points across the 8 cores
    atom_sh = atomtypes.reshape(NCORES, PTS, K, D)
    dist_sh = dist.reshape(NCORES, PTS, K, 1)

    def rep(x):  # replicate params to every core
        x = np.asarray(x, dtype=np.float32)
        return np.broadcast_to(x[None], (NCORES,) + x.shape)

    out = _run_shard(
        atom_sh, dist_sh, rep(mlp_w1), rep(mlp_b1), rep(mlp_w2), rep(mlp_b2),
        rep(gn_w), rep(gn_b),
    )
    out = np.asarray(out)  # [8, PTS, D]
    return out.reshape(B, N, D)

